# revision 13
# baseline (speedup 1.0000x reference)
"""Bass/Tile TRN2 kernel for nn_CausalAttention (softmax + tril-matmul renorm).

Math restructuring (per core, row block of B = SEQ/n_cores rows):
    q = x @ wq ; k = x @ wk ; v = x @ wv
    z = q @ k.T / sqrt(D) ;  s = exp(z)            (softmax norm cancels below)
    masked[i,j] = sum_{t>=j} s[i,t]                (suffix sum == s @ tril)
    out = (masked @ v) / rowsum(masked)

All work happens in transposed layouts ([feature/key on partitions, query on free]):
    sT[t,i] = s[i,t], computed as zT = KT-chunks.T @ qT per 128-key tile.
    masked0T_r = tril128 @ sT_r          (within-tile suffix sums, one matmul per tile)
    CS[r,i]  = colsum(sT_r)              (paired-DR selector matmuls into one PSUM tile)
    SUF[r,i] = sum_{r'>r} CS[r',i]       (one strict-tril matmul)
    numT = sum_r V_r.T-mm(masked0T_r) + VS.T-mm(SUF)     where VS[r,:] = colsum(V_r)
    den[i]  = sum_t (t+1) s[t,i]         (selector den column)
    out = (numT / den).T

Overlap structure: x.T is shipped pre-transposed from the host; the kT
projection runs first in two key-halves, each immediately AllGathered (after a
tiny warm-up collective that absorbs CC startup). V (fp8 DoubleRow) and qT
projections run under the K gathers; phase A consumes gathered K per half as it
lands; V is gathered in two feature-halves aligned with phase B's two
feature groups. Output rows are transposed back via DMA XBAR (no PE).
"""
import numpy as np
from contextlib import ExitStack

import concourse.bass as bass
import concourse.tile as tile
from concourse import bacc, mybir

F32 = mybir.dt.float32
BF16 = mybir.dt.bfloat16
FP8 = mybir.dt.float8e4
U8 = mybir.dt.uint8
AX = mybir.AxisListType
AF = mybir.ActivationFunctionType
ALU = mybir.AluOpType
DR = mybir.MatmulPerfMode.DoubleRow

P = 128
MB = 33


def make_consts(T):
    tril = np.tril(np.ones((P, P), np.float32))  # [t, j] = 1 if t >= j
    stril = np.tri(T, T, -1, dtype=np.float32)  # [r', r] = 1 if r' > r
    # Per-tile selector block [P, MB]: col rg is the colsum selector (ones),
    # col 32 the den weight column (global (t+1)/64; folded out in the output
    # scale).
    seld = np.zeros((P, T, MB), np.float32)
    for rg in range(T):
        seld[:, rg, rg] = 1.0
        seld[:, rg, 32] = (P * rg + np.arange(P) + 1.0) / 64.0
    ident = np.eye(P, dtype=np.float32)
    import ml_dtypes
    bf = lambda a: a.astype(ml_dtypes.bfloat16)
    return dict(c_tril=bf(tril), c_stril=bf(stril),
                c_seld=bf(seld.reshape(P, T * MB)),
                c_ident=ident, c_warm=np.zeros(8, np.uint8))


def build(SEQ=4096, D=1024, n_cores=8, dbg=False):
    T = SEQ // P           # global 128-key tiles
    TL = T // n_cores      # local tiles per core
    B = P * TL             # rows per core
    DC = D // P            # feature chunks
    B2 = B // 2            # key half per core
    D2 = D // 2            # feature half (phase B group width)
    H = DC // 2            # feature chunks per phase-B group
    npair = T // 2
    assert B == 512 and T <= P and TL == 4
    scale = float(1.0 / np.sqrt(D) / 64.0)   # wq,wk each prescaled x8

    nc = bacc.Bacc("TRN2", target_bir_lowering=False, debug=False, num_devices=n_cores)

    xT8_d = nc.dram_tensor("xT8", [P, DC * B], FP8, kind="ExternalInput")
    xTb_d = nc.dram_tensor("xTb", [P, DC * B], BF16, kind="ExternalInput")
    wq_d = nc.dram_tensor("wq", [D, D], FP8, kind="ExternalInput")
    wk_d = nc.dram_tensor("wk", [D, D], FP8, kind="ExternalInput")
    wv_d = nc.dram_tensor("wv", [D, D], BF16, kind="ExternalInput")
    xrs_d = nc.dram_tensor("xrsb", [DC * P, TL], BF16, kind="ExternalInput")
    c_tril = nc.dram_tensor("c_tril", [P, P], BF16, kind="ExternalInput")
    c_stril = nc.dram_tensor("c_stril", [T, T], BF16, kind="ExternalInput")
    c_seld = nc.dram_tensor("c_seld", [P, T * MB], BF16, kind="ExternalInput")
    c_ident = nc.dram_tensor("c_ident", [P, P], F32, kind="ExternalInput")
    c_warm = nc.dram_tensor("c_warm", [8], U8, kind="ExternalInput")
    out = nc.dram_tensor("out", [B, D], F32, kind="ExternalOutput")
    if dbg:
        d_qT = nc.dram_tensor("d_qT", [P, DC * B], FP8, kind="ExternalOutput")
        d_kT = nc.dram_tensor("d_kT", [P, DC * B], FP8, kind="ExternalOutput")
        d_m0 = nc.dram_tensor("d_m0", [P, 4 * B], FP8, kind="ExternalOutput")
        d_xrsg = nc.dram_tensor("d_xrsg", [P, DC * T], BF16, kind="ExternalOutput")
        d_cs = nc.dram_tensor("d_cs", [T, B], BF16, kind="ExternalOutput")
        d_suf = nc.dram_tensor("d_suf", [T, B], BF16, kind="ExternalOutput")
        d_dennat = nc.dram_tensor("d_dennat", [P, TL], F32, kind="ExternalOutput")
        d_recip = nc.dram_tensor("d_recip", [P, TL], F32, kind="ExternalOutput")
        d_vs = nc.dram_tensor("d_vs", [T, D], BF16, kind="ExternalOutput")
        d_nsb = nc.dram_tensor("d_nsb", [P, B], BF16, kind="ExternalOutput")
        d_on = nc.dram_tensor("d_on", [P, D2], BF16, kind="ExternalOutput")
        d_st = nc.dram_tensor("d_st", [P, 2 * B], BF16, kind="ExternalOutput")

    KH = D * B2            # cc1 half payload (fp8)
    VNB = B * D2           # V feature-half payload bytes (fp8)
    XRB = P * DC * TL * 2  # xrs payload bytes (bf16)

    with tile.TileContext(nc) as tc, ExitStack() as top:
        dram = top.enter_context(tc.tile_pool(name="dram", bufs=1, space="DRAM"))
        warm_in = dram.tile([8], U8)
        warm_out = dram.tile([n_cores, 8], U8, addr_space="Shared")
        cc1_in = [dram.tile([KH], FP8, name=f"cc1i{h}") for h in range(2)]
        cc1_out = [dram.tile([n_cores, KH], FP8, addr_space="Shared", name=f"cc1o{h}")
                   for h in range(2)]
        cc2a_in = dram.tile([VNB + XRB], U8)
        cc2a_out = dram.tile([n_cores, VNB + XRB], U8, addr_space="Shared")
        cc2b_in = dram.tile([VNB], U8)
        cc2b_out = dram.tile([n_cores, VNB], U8, addr_space="Shared")

        consts = top.enter_context(tc.tile_pool(name="consts", bufs=1))
        tril_sb = consts.tile([P, P], BF16)
        nc.scalar.dma_start(tril_sb[:], c_tril.ap())
        stril_sb = consts.tile([T, T], BF16)
        nc.scalar.dma_start(stril_sb[:], c_stril.ap())
        seld_sb = consts.tile([P, T * MB], BF16)
        nc.scalar.dma_start(seld_sb[:], c_seld.ap())
        ident_sb = consts.tile([P, P], F32)
        nc.scalar.dma_start(ident_sb[:], c_ident.ap())
        nc.sync.dma_start(warm_in[:], c_warm.ap())

        persist = top.enter_context(tc.tile_pool(name="persist", bufs=1))
        qT = persist.tile([P, DC * B], FP8)           # q.T row block
        kT_loc = persist.tile([P, DC * B], FP8)       # own k.T (layout [p, dc, h, 256])
        m0 = persist.tile([P, T * B], FP8)            # masked0T tiles (x 1/16)
        vs_sb = persist.tile([T, D], BF16)            # per-tile V colsums
        xrs_g = persist.tile([P, DC * T], BF16)       # gathered per-tile x row sums
        suf_sb = persist.tile([T, B], BF16)
        cs_sb = persist.tile([T, B], BF16)
        recip = persist.tile([P, TL], F32)
        dennat = persist.tile([P, TL], F32)
        den_pad = persist.tile([P, B], F32)
        wv_sb = persist.tile([P, DC * D], BF16)       # kept resident for VS
        if dbg:
            dbg_nsb = persist.tile([P, B], BF16)
            dbg_on = persist.tile([P, D2], BF16)
            dbg_st = persist.tile([P, 2 * B], BF16)

        # ------------------- stage 1: local projections (fp8 DR) -------------------
        with ExitStack() as s1:
            wpool = s1.enter_context(tc.tile_pool(name="w", bufs=1))
            xT8 = wpool.tile([P, DC * B], FP8)
            xTb = wpool.tile([P, DC * B], BF16)
            wk_sb = wpool.tile([P, DC * D], FP8)
            wq_sb = wpool.tile([P, DC * D], FP8)

            nc.sync.dma_start(xT8[:], xT8_d.ap())
            for dc in range(DC):
                nc.sync.dma_start(wk_sb[:, dc * D:(dc + 1) * D], wk_d.ap()[dc * P:(dc + 1) * P, :])
            for dc in range(DC):
                nc.scalar.dma_start(wv_sb[:, dc * D:(dc + 1) * D], wv_d.ap()[dc * P:(dc + 1) * P, :])
            nc.gpsimd.dma_start(xTb[:], xTb_d.ap())
            for dc in range(DC):
                nc.gpsimd.dma_start(wq_sb[:, dc * D:(dc + 1) * D], wq_d.ap()[dc * P:(dc + 1) * P, :])
            # warm-up collective: absorbs CC-core startup so cc1a starts hot
            # (issued after the gpsimd load issues; blocks gpsimd only)
            nc.gpsimd.collective_compute(
                "AllGather", ALU.bypass,
                replica_groups=[list(range(n_cores))],
                ins=[warm_in.opt()], outs=[warm_out.opt()],
            )
            # xrs ships straight into the cc2a staging region (DRAM->DRAM)
            nc.sync.dma_start(
                cc2a_in[VNB:VNB + XRB].rearrange("(a b) -> a b", b=2 * TL),
                xrs_d.ap().bitcast(U8))

            wk3 = wk_sb.rearrange("p (dc d) -> p dc d", dc=DC)
            wq3 = wq_sb.rearrange("p (dc d) -> p dc d", dc=DC)
            wv3 = wv_sb.rearrange("p (dc d) -> p dc d", dc=DC)
            xT83 = xT8.rearrange("p (dc b) -> p dc b", dc=DC)
            xTb3 = xTb.rearrange("p (dc b) -> p dc b", dc=DC)

            pps = s1.enter_context(tc.tile_pool(name="pps", bufs=2, space="PSUM"))

            # kT projection by key halves -> cc1a / cc1b gathers
            for h in range(2):
                for dco in range(DC):
                    k_ps = pps.tile([P, B2], F32, tag="kp", name="k_ps")
                    for pp_ in range(DC // 2):
                        nc.tensor.matmul(
                            k_ps[:],
                            wk3[:, 2 * pp_:2 * pp_ + 2, dco * P:(dco + 1) * P],
                            xT83[:, 2 * pp_:2 * pp_ + 2, h * B2:(h + 1) * B2],
                            start=(pp_ == 0), stop=(pp_ == DC // 2 - 1),
                            perf_mode=DR,
                        )
                    dst = kT_loc[:, dco * B + h * B2: dco * B + (h + 1) * B2]
                    nc.vector.tensor_copy(dst, k_ps[:])
                    nc.sync.dma_start(
                        cc1_in[h][dco * P * B2:(dco + 1) * P * B2]
                        .rearrange("(p i) -> p i", p=P),
                        dst,
                    )
                nc.gpsimd.collective_compute(
                    "AllGather", ALU.bypass,
                    replica_groups=[list(range(n_cores))],
                    ins=[cc1_in[h].opt()], outs=[cc1_out[h].opt()],
                )

            # V row block (natural layout, fp8 DR), split by feature halves
            vlp = s1.enter_context(tc.tile_pool(name="vl", bufs=3))
            for tcc in range(TL):
                for g, cc_g in enumerate([cc2a_in, cc2b_in]):
                    v_ps = pps.tile([P, D2], F32, tag="vp", name="v_ps")
                    for dci in range(DC):
                        nc.tensor.matmul(
                            v_ps[:],
                            xTb3[:, dci, tcc * P:(tcc + 1) * P],
                            wv3[:, dci, g * D2:(g + 1) * D2],
                            start=(dci == 0), stop=(dci == DC - 1),
                        )
                    vl = vlp.tile([P, D2], FP8, tag="vl")
                    nc.vector.tensor_copy(vl[:], v_ps[:])
                    nc.sync.dma_start(
                        cc_g[tcc * P * D2:(tcc + 1) * P * D2]
                        .rearrange("(p d) -> p d", p=P),
                        vl[:].bitcast(U8),
                    )

            # qT projection
            for dco in range(DC):
                q_ps = pps.tile([P, B], F32, tag="qp", name="q_ps")
                for pp_ in range(DC // 2):
                    nc.tensor.matmul(
                        q_ps[:],
                        wq3[:, 2 * pp_:2 * pp_ + 2, dco * P:(dco + 1) * P],
                        xT83[:, 2 * pp_:2 * pp_ + 2, :],
                        start=(pp_ == 0), stop=(pp_ == DC // 2 - 1),
                        perf_mode=DR,
                    )
                nc.vector.tensor_copy(qT[:, dco * B:(dco + 1) * B], q_ps[:])

        # V + xrs gathers (queue behind cc1a/cc1b on the CC cores)
        nc.gpsimd.collective_compute(
            "AllGather", ALU.bypass,
            replica_groups=[list(range(n_cores))],
            ins=[cc2a_in.opt()], outs=[cc2a_out.opt()],
        )
        nc.gpsimd.collective_compute(
            "AllGather", ALU.bypass,
            replica_groups=[list(range(n_cores))],
            ins=[cc2b_in.opt()], outs=[cc2b_out.opt()],
        )

        # ------------------- phase A: scores / exp / per-tile sums -------------------
        qT3 = qT.rearrange("p (dc b) -> p dc b", dc=DC)
        with ExitStack() as pa:
            ktp = pa.enter_context(tc.tile_pool(name="kt", bufs=3))
            stp = pa.enter_context(tc.tile_pool(name="st", bufs=3))
            ztp = pa.enter_context(tc.tile_pool(name="zt", bufs=2, space="PSUM"))
            mtp = pa.enter_context(tc.tile_pool(name="mt", bufs=2, space="PSUM"))
            csp = pa.enter_context(tc.tile_pool(name="csp", bufs=1, space="PSUM"))
            sfp = pa.enter_context(tc.tile_pool(name="sfp", bufs=1, space="PSUM"))
            cs_ps = csp.tile([MB, B], F32)

            for h in range(2):
                for rc in range(n_cores):
                    ktc = ktp.tile([P, DC * B2], FP8, tag="kt")
                    for dc in range(DC):
                        nc.sync.dma_start(
                            ktc[:, dc * B2:(dc + 1) * B2],
                            cc1_out[h][rc, dc * P * B2:(dc + 1) * P * B2]
                            .rearrange("(p i) -> p i", p=P),
                        )
                    ktc3 = ktc.rearrange("p (dc i) -> p dc i", dc=DC)
                    st_pair = stp.tile([P, 2 * B], BF16, tag="st")
                    for sub2 in range(2):
                        rg = rc * TL + h * 2 + sub2
                        first = (h == 0) and (rc == 0) and (sub2 == 0)
                        last = (h == 1) and (rc == n_cores - 1) and (sub2 == 1)
                        zt = ztp.tile([P, B], F32, tag="zt")
                        for pp in range(DC // 2):
                            nc.tensor.matmul(
                                zt[:],
                                ktc3[:, 2 * pp:2 * pp + 2, sub2 * P:(sub2 + 1) * P],
                                qT3[:, 2 * pp:2 * pp + 2, :],
                                start=(pp == 0), stop=(pp == DC // 2 - 1),
                                perf_mode=DR,
                            )
                        sth = st_pair[:, sub2 * B:(sub2 + 1) * B]
                        nc.scalar.activation(sth, zt[:], AF.Exp, scale=scale)
                        mt = mtp.tile([P, B], F32, tag="mt")
                        nc.tensor.matmul(mt[:], tril_sb[:], sth, start=True, stop=True)
                        if rg % 2 == 0:
                            nc.vector.tensor_scalar(
                                m0[:, rg * B:(rg + 1) * B], mt[:], 0.0625, None,
                                op0=ALU.mult)
                        else:
                            nc.scalar.activation(
                                m0[:, rg * B:(rg + 1) * B], mt[:], AF.Copy,
                                scale=0.0625)
                        nc.tensor.matmul(
                            cs_ps[:], seld_sb[:, rg * MB:(rg + 1) * MB], sth,
                            start=first, stop=last,
                        )
                    if dbg and h == 0 and rc == 0:
                        nc.vector.tensor_copy(dbg_st[:], st_pair[:])

            nc.vector.tensor_copy(cs_sb[:], cs_ps[0:T, :])
            nc.vector.memset(den_pad[:], 0.0)
            nc.vector.tensor_copy(den_pad[32:33, :], cs_ps[32:33, :])
            suf_ps = sfp.tile([T, B], F32, tag="sf")
            nc.tensor.matmul(suf_ps[:], stril_sb[:], cs_sb[:], start=True, stop=True)
            nc.vector.tensor_scalar(suf_sb[:], suf_ps[:], 0.0625, None, op0=ALU.mult)

        # ------------------- phase B: VS, den, numT, output -------------------
        with ExitStack() as pb:
            trp2 = pb.enter_context(tc.tile_pool(name="trp2", bufs=2, space="PSUM"))
            vsps = pb.enter_context(tc.tile_pool(name="vsps", bufs=2, space="PSUM"))
            vrp = pb.enter_context(tc.tile_pool(name="vr", bufs=4))
            outp = pb.enter_context(tc.tile_pool(name="outp", bufs=3))
            otn = pb.enter_context(tc.tile_pool(name="otn", bufs=3))
            nump = pb.enter_context(tc.tile_pool(name="nump", bufs=H, space="PSUM"))
            nsbp = pb.enter_context(tc.tile_pool(name="nsb", bufs=H))

            # 1/den (overlaps VS + the first group's matmuls)
            for sub in range(TL):
                dps = trp2.tile([P, P], F32, tag="tr2")
                nc.tensor.transpose(dps[:], den_pad[:, sub * P:(sub + 1) * P], ident_sb[:])
                nc.vector.tensor_copy(dennat[:, sub:sub + 1], dps[:, 32:33])
            nc.vector.reciprocal(recip[:], dennat[:])

            # gathered x row sums -> VS = xrs.T-mm(wv)  [T, D] (after cc2a)
            for dc in range(DC):
                nc.sync.dma_start(
                    xrs_g[:, dc * T:(dc + 1) * T].bitcast(U8),
                    cc2a_out[0:n_cores, VNB + dc * P * TL * 2: VNB + (dc + 1) * P * TL * 2]
                    .rearrange("c (p t) -> p c t", p=P),
                )
            xrs3 = xrs_g.rearrange("p (dc t) -> p dc t", dc=DC)
            for nh in range(2):
                vs_ps = vsps.tile([T, D2], F32, tag="vs")
                for dci in range(DC):
                    nc.tensor.matmul(
                        vs_ps[:],
                        xrs3[:, dci, :],
                        wv3[:, dci, nh * D2:(nh + 1) * D2],
                        start=(dci == 0), stop=(dci == DC - 1),
                    )
                nc.vector.tensor_copy(vs_sb[:, nh * D2:(nh + 1) * D2], vs_ps[:])

            for g, cc_g in enumerate([cc2a_out, cc2b_out]):
                nums = [nump.tile([P, B], F32, tag="num", name=f"num_ps{g}_{i}")
                        for i in range(H)]
                # fp8 DoubleRow: two 128-key tiles per matmul (contraction 256).
                # V-pairs OPEN the psum group and the VS x SUF term CLOSES it.
                for rc in range(n_cores):
                    for pr in range(TL // 2):
                        rg = rc * TL + 2 * pr
                        vp = vrp.tile([P, 2 * D2], FP8, tag="vr")
                        for t_ in range(2):
                            nc.scalar.dma_start(
                                vp[:, t_ * D2:(t_ + 1) * D2].bitcast(U8),
                                cc_g[rc, (2 * pr + t_) * P * D2:(2 * pr + t_ + 1) * P * D2]
                                .rearrange("(p d) -> p d", p=P),
                            )
                        vp3 = vp.rearrange("p (two n) -> p two n", two=2)
                        m3 = m0[:, rg * B:(rg + 2) * B].rearrange("p (two b) -> p two b", two=2)
                        for i in range(H):
                            nc.tensor.matmul(
                                nums[i][:], vp3[:, :, i * P:(i + 1) * P], m3,
                                start=(rg == 0), stop=False,
                                perf_mode=DR,
                            )
                for i in range(H):
                    dc2 = g * H + i
                    nc.tensor.matmul(
                        nums[i][:], vs_sb[:, dc2 * P:(dc2 + 1) * P], suf_sb[:],
                        start=False, stop=True,
                    )
                # group epilogue: PSUM -> bf16, DMA-XBAR transpose, scale, store
                num_sb = []
                _dbg_here = dbg and g == 0
                for i in range(H):
                    t_ = nsbp.tile([P, B], BF16, tag="nsb", name=f"num_sb{g}_{i}")
                    (nc.vector.tensor_copy if i % 2 == 0 else
                     (lambda d, s: nc.scalar.activation(d, s, AF.Copy)))(t_[:], nums[i][:])
                    num_sb.append(t_)
                if _dbg_here:
                    nc.vector.tensor_copy(dbg_nsb[:], num_sb[0][:])
                for sub in range(TL):
                    ot = outp.tile([P, D2], F32, tag="ot")
                    on = otn.tile([P, D2], BF16, tag="on")
                    for i in range(H):
                        (nc.sync if i % 2 == 0 else nc.scalar).dma_start_transpose(
                            on[:, i * P:(i + 1) * P],
                            num_sb[i][:, sub * P:(sub + 1) * P],
                        )
                        nc.gpsimd.tensor_scalar(
                            ot[:, i * P:(i + 1) * P], on[:, i * P:(i + 1) * P],
                            recip[:, sub:sub + 1], 0.25,
                            op0=ALU.mult, op1=ALU.mult,
                        )
                    if _dbg_here and sub == 0:
                        nc.vector.tensor_copy(dbg_on[:], on[:])
                    nc.sync.dma_start(
                        out.ap()[sub * P:(sub + 1) * P, g * D2:(g + 1) * D2],
                        ot[:],
                    )

        if dbg:
            nc.sync.dma_start(d_qT.ap(), qT[:])
            nc.sync.dma_start(d_kT.ap(), kT_loc[:])
            nc.sync.dma_start(d_m0.ap(), m0[:, 0:4 * B])
            nc.sync.dma_start(d_xrsg.ap(), xrs_g[:])
            nc.sync.dma_start(d_cs.ap(), cs_sb[:])
            nc.sync.dma_start(d_suf.ap(), suf_sb[:])
            nc.sync.dma_start(d_dennat.ap(), dennat[:])
            nc.sync.dma_start(d_recip.ap(), recip[:])
            nc.sync.dma_start(d_vs.ap(), vs_sb[:])
            nc.sync.dma_start(d_nsb.ap(), dbg_nsb[:])
            nc.sync.dma_start(d_on.ap(), dbg_on[:])
            nc.sync.dma_start(d_st.ap(), dbg_st[:])

    nc.compile()
    return nc


def make_in_maps(x_full, wq, wk, wv, n_cores=8):
    import ml_dtypes
    f8 = lambda a: np.ascontiguousarray(a).astype(ml_dtypes.float8_e4m3)
    bf = lambda a: np.ascontiguousarray(a).astype(ml_dtypes.bfloat16)
    SEQ, D = x_full.shape
    T = SEQ // P
    TL = T // n_cores
    B = SEQ // n_cores
    DC = D // P
    consts = make_consts(T)
    # wq/wk prescaled x8 (folded back out in the exp scale)
    wq8, wk8, wvb = f8(wq * 8.0), f8(wk * 8.0), bf(wv)
    in_maps = []
    for c in range(n_cores):
        xs = x_full[c * B:(c + 1) * B]                    # [B, D]
        # xT[p, dc*B + i] = x[i, dc*128 + p]
        xT = np.transpose(xs.reshape(B, DC, P), (2, 1, 0)).reshape(P, DC * B)
        # xrs flat order [dc, p, t]: row sums of x per local tile
        xrs = xs.reshape(TL, P, DC, P).sum(axis=1)        # [TL, DC, P]
        xrs = np.transpose(xrs, (1, 2, 0)).reshape(DC * P, TL)
        m = {"xT8": f8(xT), "xTb": bf(xT), "xrsb": bf(xrs),
             "wq": wq8, "wk": wk8, "wv": wvb}
        m.update(consts)
        in_maps.append(m)
    return in_maps


def algo_ref(x, wq, wk, wv):
    """Numpy float64 reference of the restructured math (for validation)."""
    x = x.astype(np.float64)
    q = x @ wq.astype(np.float64)
    k = x @ wk.astype(np.float64)
    v = x @ wv.astype(np.float64)
    z = q @ k.T / np.sqrt(k.shape[1])
    s = np.exp(z)
    masked = np.cumsum(s[:, ::-1], axis=1)[:, ::-1]
    num = masked @ v
    den = masked.sum(axis=1)
    return (num / den[:, None]).astype(np.float32)



# ----------------------------------------------------------------------------
# Harness entry point: full (unsharded) inputs -> full output.
# ----------------------------------------------------------------------------
SEQ, D_IN, N_CORES = 4096, 1024, 8
_built = {}


def _get_nc(SEQ_=SEQ, D_=D_IN, n_cores=N_CORES):
    key = (SEQ_, D_, n_cores)
    if key not in _built:
        _built[key] = build(SEQ=SEQ_, D=D_, n_cores=n_cores)
    return _built[key]


def run(x, wq, wk, wv, trace=False, **spmd_kwargs):
    from concourse.bass_utils import run_bass_kernel_spmd

    x = np.ascontiguousarray(np.asarray(x, dtype=np.float32))
    wq = np.ascontiguousarray(np.asarray(wq, dtype=np.float32))
    wk = np.ascontiguousarray(np.asarray(wk, dtype=np.float32))
    wv = np.ascontiguousarray(np.asarray(wv, dtype=np.float32))
    n_cores = N_CORES
    nc = _get_nc(x.shape[0], x.shape[1], n_cores)
    in_maps = make_in_maps(x, wq, wk, wv, n_cores=n_cores)
    res = run_bass_kernel_spmd(nc, in_maps, list(range(n_cores)),
                               trace=trace, **spmd_kwargs)
    out = np.concatenate([res.results[c]["out"] for c in range(n_cores)], axis=0)
    return out, res


def kernel(x, wq, wk, wv):
    out, _ = run(x, wq, wk, wv, trace=False)
    return out
